# revision 24
# baseline (speedup 1.0000x reference)
"""Trainium2 Bass kernel for nn_CantorModalityFusion.

Sharding: 8 cores = (batch b in 0..3) x (position parity in 0..1).
Each core handles batch b, positions s = par, par+2, ... (1024 positions).
The computation is per-(b, s) independent -> no collectives.

Key structure (v2, folded):
  - Q/K/V only ever consume the modality projections p_m = W_m x + c_m,
    so for modalities with dim_m <= 1.5*D the host folds (Wq @ W_m) etc.
    and the kernel computes q/k/v straight from x (skips stage A).
    Video (dim 2048) keeps the two-stage path.  Folded biases
    Wq(b_m + emb_m) + bq are precomputed per (mat, modality).
  - fp16 weights/activations with fp32 PSUM accumulation: PE rate is
    unchanged (1 cycle/row), DMA/SBUF traffic halves, DVE gets 2x.
  - Matmul free dims restricted to each modality's active positions.
  - Folded text/audio weights are iteration-invariant SBUF residents.

Pipeline per 512-position block:
  [blk0] pT_video = Wv.T @ x_video (+bias)                     [PE+ACT]
  P1: qT/kT per feature chunk from x (folded) / pT (video);
      s_w += selw.T @ (q*k)                                    [PE+ACT+DVE]
  SM: softmax over the 3 routed windows                        [DVE+ACT]
  P2: vT per chunk; A16_r = sum attn; Abc = bcast(A16);
      fz[c] = sum_r Abc_r * v_r                                [PE+DVE]
  D:  y.T = Wo.T.T @ fz (+ bo)  (deferred one block)           [PE+ACT]
"""

import sys

import numpy as np

sys.path.insert(0, "/opt/trn_rl_repo")

import concourse.bacc as bacc
import concourse.mybir as mybir
from concourse import tile
from concourse.bass_utils import run_bass_kernel_spmd

F32 = mybir.dt.float32
F16 = mybir.dt.float16
AF = mybir.ActivationFunctionType
ALU = mybir.AluOpType

B, S, D, H, HD = 4, 2048, 1024, 16, 64
M, WIN = 4, 3
NAMES = ["text", "image", "audio", "video"]
DIMS = [768, 1024, 512, 2048]
NK = [d // 128 for d in DIMS]            # 6, 8, 4, 16
ROUTES = [[0, 1, 2], [0, 1, 2], [2, 3, 0], [3, 2, 0]]
PAIRS = [(m, w, ROUTES[m][w]) for m in range(M) for w in range(WIN)]
SRC = {r: [(m, w) for (m, w, rr) in PAIRS if rr == r] for r in range(M)}
PAIR_IDX = {(m, w): m * WIN + w for m in range(M) for w in range(WIN)}
FOLD = (0, 1, 2)                         # folded modalities (video = 3 is 2-stage)

NPOS = S // 2
BLK = 512
NBLK = NPOS // BLK
NCH = D // 128                           # 8 feature chunks
NLOC = [sl // 2 for sl in (2048, 1024, 1500, 512)]   # 1024, 512, 750, 256

_BUILD_CACHE = {}


def n_active(m, blk):
    return max(0, min(BLK, NLOC[m] - blk * BLK))


def build(scale, repeat=1):
    key = (float(scale), repeat)
    if key in _BUILD_CACHE:
        return _BUILD_CACHE[key]
    nc = bacc.Bacc("TRN2", target_bir_lowering=False, debug=False)

    xT = [nc.dram_tensor(f"xT{m}", [128, NK[m], NLOC[m]], F16, kind="ExternalInput")
          for m in range(M)]
    # folded q/k/v weights for text/image/audio: [p, mat, c, dk, j]
    F_d = [nc.dram_tensor(f"F{m}", [128, 3, NCH, NK[m], 128], F16,
                          kind="ExternalInput") for m in FOLD]
    Wvid_d = nc.dram_tensor("Wvid", [128, NCH, NK[3], 128], F16, kind="ExternalInput")
    Fvid_d = nc.dram_tensor("Fvid", [128, 3, NCH, NCH, 128], F16, kind="ExternalInput")
    WoT_d = nc.dram_tensor("WoT", [128, NCH, NCH, 128], F16, kind="ExternalInput")
    # bias slots: mat*3+m (folded qkv, m in 0..2); 9 video stage-A;
    # 10+mat video qkv; 13 bo
    biasC_d = nc.dram_tensor("biasC", [128, 14, NCH], F32, kind="ExternalInput")
    selw_d = nc.dram_tensor("selw", [128, 127], F16, kind="ExternalInput")
    selA_d = nc.dram_tensor("selA", [64, M * WIN, 16], F16, kind="ExternalInput")
    selB_d = nc.dram_tensor("selB", [16, NCH, 128], F16, kind="ExternalInput")
    yT_d = nc.dram_tensor("yT", [NCH, 128, NPOS], F16, kind="ExternalOutput")

    with tile.TileContext(nc) as tc:
        with (
            tc.tile_pool(name="const", bufs=1) as cpool,
            tc.tile_pool(name="wst", bufs=2) as wpool,
            tc.tile_pool(name="x2", bufs=2) as xpool2,
            tc.tile_pool(name="x1", bufs=1) as xpool1,
            tc.tile_pool(name="qk", bufs=1) as qkpool,
            tc.tile_pool(name="pr", bufs=2) as prpool,
            tc.tile_pool(name="sm", bufs=1) as smpool,
            tc.tile_pool(name="pt", bufs=1) as ptpool,
            tc.tile_pool(name="fz", bufs=1) as fzpool,
            tc.tile_pool(name="yo", bufs=2) as yopool,
            tc.tile_pool(name="ps", bufs=1, space="PSUM") as pspool,
        ):
            def ps(i, shape=(128, BLK)):
                return pspool.tile(list(shape), F32, tag=f"a{i}", name=f"ps_a{i}")

            # ---- constants + resident weights ----
            selw = cpool.tile([128, 127], F16, tag="selw")
            nc.sync.dma_start(selw[:], selw_d[:])
            selA = cpool.tile([64, M * WIN, 16], F16, tag="selA")
            nc.sync.dma_start(selA[:], selA_d[:])
            selB = cpool.tile([16, NCH, 128], F16, tag="selB")
            nc.sync.dma_start(selB[:], selB_d[:])
            biasC = cpool.tile([128, 14, NCH], F32, tag="biasC")
            nc.sync.dma_start(biasC[:], biasC_d[:])
            Fres = {}
            for m in (0, 2):  # text + audio resident (used in both blocks)
                t = cpool.tile([128, 3, NCH, NK[m], 128], F16, tag=f"Fres{m}")
                nc.sync.dma_start(t[:], F_d[FOLD.index(m)][:])
                Fres[m] = t

            import contextlib
            rep_cm = (tc.For_i(0, repeat, 1,
                               hint_engines=(mybir.EngineType.PE,
                                             mybir.EngineType.Activation,
                                             mybir.EngineType.DVE,
                                             mybir.EngineType.SP,
                                             mybir.EngineType.Pool))
                      if repeat > 1 else contextlib.nullcontext())

            pending_D = []
            rot = {"qk": 0, "v": 0, "ab": 0}

            def emit_stage_D(fz, p0):
                for dc in range(NCH):
                    wo = wpool.tile([128, NCH, 128], F16, tag="wo", name="wo")
                    nc.gpsimd.dma_start(wo[:], WoT_d[:, dc])
                    acc = ps(3 + dc % 2)
                    for dk in range(NCH):
                        nc.tensor.matmul(
                            acc[:], wo[:, dk, :], fz[:, dk, :],
                            start=(dk == 0), stop=(dk == NCH - 1),
                            skip_group_check=True)
                    yo = yopool.tile([128, BLK], F16, tag="yo")
                    nc.scalar.activation(yo[:], acc[:], AF.Identity,
                                         bias=biasC[:, 13, dc:dc + 1])
                    nc.gpsimd.dma_start(yT_d[dc, :, p0:p0 + BLK], yo[:])

            with rep_cm:
                for blk in range(NBLK):
                    p0 = blk * BLK
                    nact = [n_active(m, blk) for m in range(M)]
                    act_m = [m for m in range(M) if nact[m] > 0]
                    act_pairs = [(m, w, r, min(nact[m], nact[r]))
                                 for (m, w, r) in PAIRS
                                 if nact[m] > 0 and nact[r] > 0]

                    # ---- x DMAs ----
                    xt = {}
                    for m in act_m:
                        pool = xpool2 if m in (0, 2) else xpool1
                        na = nact[m]
                        wid = BLK if m != 3 else 256
                        t = pool.tile([128, NK[m], wid], F16, tag=f"x{m}",
                                      name=f"x{m}")
                        nc.gpsimd.dma_start(t[:, :, :na],
                                            xT[m][:, :, p0:p0 + na])
                        xt[m] = t

                    # ---- video stage A (blk0 only) ----
                    if nact[3] > 0:
                        ptv = ptpool.tile([128, NCH, 256], F16, tag="ptv",
                                          name="ptv")
                        for c in range(NCH):
                            wva = wpool.tile([128, NK[3], 128], F16, tag="wva",
                                             name="wva")
                            nc.scalar.dma_start(wva[:], Wvid_d[:, c])
                            acc = ps(c % 3, (128, 256))
                            for dk in range(NK[3]):
                                nc.tensor.matmul(
                                    acc[:], wva[:, dk, :], xt[3][:, dk, :],
                                    start=(dk == 0), stop=(dk == NK[3] - 1),
                                    skip_group_check=True)
                            nc.scalar.activation(ptv[:, c, :], acc[:],
                                                 AF.Identity,
                                                 bias=biasC[:, 9, c:c + 1])

                    # ---- pass 1: q, k, scores ----
                    wcount = {w: 0 for w in range(WIN)}
                    n_sc = {w: sum(1 for (m, w2, r, f) in act_pairs
                                   if w2 == w) * NCH for w in range(WIN)}
                    sc_ps = [ps(5 + w, (64, BLK)) if n_sc[w] > 0 else None
                             for w in range(WIN)]

                    def qkv_mm(mat, m, c, na, acc):
                        """Accumulate q/k/v chunk c of modality m into acc."""
                        if m == 3:
                            wv = wpool.tile([128, NCH, 128], F16,
                                            tag=f"wv{mat}", name="wv")
                            nc.sync.dma_start(wv[:], Fvid_d[:, mat, c])
                            for dk in range(NCH):
                                nc.tensor.matmul(
                                    acc[:, :na], wv[:, dk, :], ptv[:, dk, :],
                                    start=(dk == 0), stop=(dk == NCH - 1),
                                    skip_group_check=True)
                        else:
                            if m == 1:
                                st = wpool.tile([128, NK[1], 128], F16,
                                                tag=f"wim{mat}", name="wim")
                                nc.scalar.dma_start(st[:], F_d[1][:, mat, c])
                            else:
                                st = Fres[m][:, mat, c]
                            for dk in range(NK[m]):
                                nc.tensor.matmul(
                                    acc[:, :na], st[:, dk, :],
                                    xt[m][:, dk, :na],
                                    start=(dk == 0), stop=(dk == NK[m] - 1),
                                    skip_group_check=True)

                    def emit_qk(c):
                        out = {}
                        for mat, tname in ((0, "q"), (1, "k")):
                            for m in act_m:
                                na = nact[m]
                                acc = ps(rot["qk"] % 5)
                                rot["qk"] += 1
                                qkv_mm(mat, m, c, na, acc)
                                t = qkpool.tile([128, BLK], F16,
                                                tag=f"{tname}{m}_{c % 3}",
                                                name=f"{tname}{m}")
                                if m == 3:
                                    nc.vector.tensor_scalar_add(
                                        t[:, :na], acc[:, :na],
                                        biasC[:, 10 + mat, c:c + 1])
                                else:
                                    nc.scalar.activation(
                                        t[:, :na], acc[:, :na], AF.Identity,
                                        bias=biasC[:, mat * 3 + m, c:c + 1])
                                out[(tname, m)] = t
                        return out

                    def emit_scores(c, qk_t):
                        for pi, (m, w, r, f) in enumerate(act_pairs):
                            prod = prpool.tile([128, BLK], F16, bufs=1,
                                               tag=f"pr{pi % 4}", name="prod")
                            nc.vector.tensor_mul(prod[:, :f],
                                                 qk_t[("q", m)][:, :f],
                                                 qk_t[("k", r)][:, :f])
                            off = 62 - (16 * m + 2 * c)
                            i = wcount[w]
                            wcount[w] += 1
                            nc.tensor.matmul(
                                sc_ps[w][:, :f], selw[:, off:off + 64],
                                prod[:, :f],
                                start=(i == 0), stop=(i == n_sc[w] - 1),
                                skip_group_check=True)

                    qkbuf = {0: emit_qk(0), 1: emit_qk(1)}
                    for c in range(2, NCH):
                        qkbuf[c] = emit_qk(c)
                        emit_scores(c - 2, qkbuf.pop(c - 2))
                    emit_scores(NCH - 2, qkbuf.pop(NCH - 2))
                    emit_scores(NCH - 1, qkbuf.pop(NCH - 1))
                    if pending_D:
                        emit_stage_D(*pending_D.pop(0))

                    # ---- softmax ----
                    fmax = {w: max([f for (m2, w2, r2, f) in act_pairs
                                    if w2 == w], default=0)
                            for w in range(WIN)}
                    s_sb, e_sb = [], []
                    for w in range(WIN):
                        t = smpool.tile([64, BLK], F32, tag=f"s{w}")
                        if sc_ps[w] is None:
                            nc.gpsimd.memset(t[:], 0.0)
                        elif fmax[w] < BLK:
                            nc.gpsimd.memset(t[:], 0.0)
                            nc.vector.tensor_copy(t[:, :fmax[w]],
                                                  sc_ps[w][:, :fmax[w]])
                        else:
                            nc.vector.tensor_copy(t[:], sc_ps[w][:])
                        s_sb.append(t)
                    mx = smpool.tile([64, BLK], F32, tag="mx")
                    nc.vector.tensor_tensor(mx[:], s_sb[0][:], s_sb[1][:],
                                            op=ALU.max)
                    nc.vector.tensor_tensor(mx[:], mx[:], s_sb[2][:],
                                            op=ALU.max)
                    for w in range(WIN):
                        nc.vector.tensor_tensor(s_sb[w][:], s_sb[w][:], mx[:],
                                                op=ALU.subtract)
                        e = smpool.tile([64, BLK], F32, tag=f"e{w}")
                        nc.scalar.activation(e[:], s_sb[w][:], AF.Exp,
                                             scale=scale)
                        e_sb.append(e)
                    den = smpool.tile([64, BLK], F32, tag="mx")
                    nc.vector.tensor_add(den[:], e_sb[0][:], e_sb[1][:])
                    nc.vector.tensor_add(den[:], den[:], e_sb[2][:])
                    rec = smpool.tile([64, BLK], F32, tag="rec")
                    with nc.allow_low_precision(reason="fp16 attn weights"):
                        nc.vector.reciprocal(rec[:], den[:])
                    attn = []
                    for w in range(WIN):
                        a = smpool.tile([64, BLK], F16, tag=f"at{w}")
                        nc.vector.tensor_mul(a[:], e_sb[w][:], rec[:])
                        attn.append(a)

                    # ---- pass 2: v, A16, Abc, fused ----
                    act_r = [r for r in range(M) if nact[r] > 0]
                    fz = fzpool.tile([128, NCH, BLK], F16, tag="fz", name="fz")

                    def emit_v(c):
                        v_t = {}
                        for m in act_m:
                            na = nact[m]
                            acc = ps(5 + rot["v"] % 3)
                            rot["v"] += 1
                            qkv_mm(2, m, c, na, acc)
                            t = qkpool.tile([128, BLK], F16,
                                            tag=f"q{m}_{c % 3}", name=f"v{m}")
                            slot = 6 + m if m != 3 else 12
                            nc.vector.tensor_scalar_add(
                                t[:, :na], acc[:, :na],
                                biasC[:, slot, c:c + 1])
                            if na < BLK:
                                nc.gpsimd.memset(t[:, na:], 0.0)
                            v_t[m] = t
                        return v_t

                    vbuf = {0: emit_v(0)}
                    if NCH > 1:
                        vbuf[1] = emit_v(1)

                    a16sb = smpool.tile([16, M, BLK], F16, tag="a16sb")
                    for r in act_r:
                        a16 = ps(r % 2, (16, BLK))
                        srcs = SRC[r]
                        for i, (m, w) in enumerate(srcs):
                            nc.tensor.matmul(
                                a16[:], selA[:, PAIR_IDX[(m, w)], :],
                                attn[w][:],
                                start=(i == 0), stop=(i == len(srcs) - 1),
                                skip_group_check=True)
                        nc.scalar.activation(a16sb[:, r, :], a16[:],
                                             AF.Identity)

                    for c in range(NCH):
                        v_t = vbuf.pop(c)
                        ab_ps = {}
                        for r in act_r:
                            ab = ps(rot["ab"] % 3)
                            rot["ab"] += 1
                            nc.tensor.matmul(ab[:], selB[:, c, :],
                                             a16sb[:, r, :],
                                             start=True, stop=True,
                                             skip_group_check=True)
                            ab_ps[r] = ab
                        r0 = act_r[0]
                        accv = prpool.tile([128, BLK], F16, tag="fa",
                                           name="accv")
                        nc.vector.tensor_mul(accv[:], ab_ps[r0][:],
                                             v_t[r0][:])
                        if len(act_r) == 1:
                            nc.vector.tensor_copy(fz[:, c, :], accv[:])
                        for j, r in enumerate(act_r[1:]):
                            tmp = prpool.tile([128, BLK], F16, tag="fb",
                                              name="tmp")
                            nc.vector.tensor_mul(tmp[:], ab_ps[r][:],
                                                 v_t[r][:])
                            last = (j == len(act_r) - 2)
                            nc.vector.tensor_add(
                                fz[:, c, :] if last else accv[:],
                                accv[:], tmp[:])
                        if c + 2 < NCH:
                            vbuf[c + 2] = emit_v(c + 2)

                    # ---- stage D (deferred one block) ----
                    pending_D.append((fz, p0))
                    if blk == NBLK - 1:
                        while pending_D:
                            emit_stage_D(*pending_D.pop(0))

    nc.compile()
    _BUILD_CACHE[key] = nc
    return nc


def make_selw():
    sw = np.zeros((128, 127), np.float32)
    for p in range(128):
        sw[p, 62 + p // 64] = 1.0
    return sw


def make_selA():
    sa = np.zeros((64, M * WIN, 16), np.float32)
    for m in range(M):
        for w in range(WIN):
            for h in range(16):
                sa[16 * m + h, m * WIN + w, h] = 1.0
    return sa


def make_selB():
    sb = np.zeros((16, NCH, 128), np.float32)
    for c in range(NCH):
        for j in range(128):
            sb[2 * c + j // 64, c, j] = 0.25
    return sb


def _vec_tile(v):
    return np.ascontiguousarray(np.asarray(v, np.float32).reshape(NCH, 128).T)


def _wlayout(WT, nk):
    """[din, dout] -> [p, c, dk, j] with din = nk*128, dout = NCH*128."""
    return np.ascontiguousarray(
        WT.reshape(nk, 128, NCH, 128).transpose(1, 2, 0, 3))


def prepare_in_maps(inputs):
    f32, f16 = np.float32, np.float16
    Wm = [np.asarray(inputs[f"W_{nm}"], f32) for nm in NAMES]
    bm = [np.asarray(inputs[f"b_{nm}"], f32) for nm in NAMES]
    emb = np.asarray(inputs["mod_emb"], f32)
    Wq, Wk, Wv, Wo = (np.asarray(inputs[f"W{x}"], f32) for x in "qkvo")
    bq, bk, bv, bo = (np.asarray(inputs[f"b{x}"], f32) for x in "qkvo")
    mats = ((Wq, bq), (Wk, bk), (Wv, bv))

    shared = {}
    for m in FOLD:
        folded = [_wlayout((Wmat @ Wm[m]).T, NK[m]) for (Wmat, _) in mats]
        shared[f"F{m}"] = np.ascontiguousarray(
            np.stack(folded, axis=1), f16)                 # [p, mat, c, dk, j]
    shared["Wvid"] = _wlayout(Wm[3].T, NK[3]).astype(f16)
    shared["Fvid"] = np.ascontiguousarray(
        np.stack([_wlayout(Wmat.T, NCH) for (Wmat, _) in mats], axis=1), f16)
    shared["WoT"] = _wlayout(Wo.T, NCH).astype(f16)

    bias = np.zeros((128, 14, NCH), f32)
    for m in FOLD:
        base = bm[m] + emb[m]
        for mat_i, (Wmat, bmat) in enumerate(mats):
            bias[:, mat_i * 3 + m, :] = _vec_tile(Wmat @ base + bmat)
    bias[:, 9, :] = _vec_tile(bm[3] + emb[3])
    for mat_i, (_, bmat) in enumerate(mats):
        bias[:, 10 + mat_i, :] = _vec_tile(bmat)
    bias[:, 13, :] = _vec_tile(bo)
    shared["biasC"] = bias

    shared["selw"] = make_selw().astype(f16)
    shared["selA"] = make_selA().astype(f16)
    shared["selB"] = make_selB().astype(f16)

    in_maps = []
    for core in range(8):
        b, par = core // 2, core % 2
        im = dict(shared)
        for i, nm in enumerate(NAMES):
            x = np.asarray(inputs[nm], f32)[b, par::2][:NLOC[i]]
            xt = np.ascontiguousarray(x.T).astype(f16)     # [dim, NLOC]
            im[f"xT{i}"] = np.ascontiguousarray(
                xt.reshape(NK[i], 128, NLOC[i]).transpose(1, 0, 2))
        in_maps.append(im)
    return in_maps


def kernel(**inputs):
    inputs = {k: np.asarray(v) for k, v in inputs.items()}
    scale = float(1.0 / (np.sqrt(HD) * abs(float(inputs["temperature"]))))
    nc = build(scale, repeat=1)
    in_maps = prepare_in_maps(inputs)
    res = run_bass_kernel_spmd(nc, in_maps, list(range(8)))
    out = np.zeros((B, S, D), np.float32)
    for core in range(8):
        b, par = core // 2, core % 2
        y = np.asarray(res.results[core]["yT"], np.float32).reshape(D, NPOS)
        out[b, par::2, :] = y.T
    return out
